# revision 14
# baseline (speedup 1.0000x reference)
"""AttentionAggregation kernel for 8 TRN2 NeuronCores (v2 restructure).

Math: out[b] = mean_n softmax(Q K^T)[n,:] @ V  with Q/K/V = x @ W^T + b.
Fold: out[b,d] = sum_m w[b,m] V[b,m,d],  w[b,m] = (1/N) sum_n E[n,m]/R[n],
E = exp(S - c), R[n] = rowsum E.  Fixed shift c sized so E fits fp8e5 range.
V is never materialized: out = (w @ x) @ Wv^T + 0.5*bv (since sum_m w[m] = 1/2
per core), so the V projection and its eviction disappear from the main loop.

Structure per core (batch b = c//2, row half h = c%2, 16 row tiles of 128):
- PSUM: banks 0-5 = three 1024-col slots for S groups; banks 6-7 = colsum
  accumulator, live across all 8 pairs (no per-pair evictions).  Slot
  rotation alternates r in {0,1} per tile: writes (g0,g1,g2,g3) ->
  (r, r+1, r+2 mod 3, r); the ACT exp pair always sees a contiguous
  ascending 2048-col window and every slot's next writer follows the
  reader that frees it.
- Exp split: groups 0-1 on ACT (one 2048-wide exp + accum for R), groups
  2-3 Schraudolph on DVE (int8-bitcast e5m2 bits; host pre-scales Wq/bq by
  A8=4/ln2), row-sum re-read of groups 2-3 on Pool (one 2048-wide
  tensor_reduce into an R cell).  Slots only ever hold S'/projection values
  (<= ~117 in e5m2-bit units), so a stale int8 convert can never saturate
  into the e5m2 NaN code 127.
- rr = 1/R per pair: one reduce + reciprocal + fp8 cast writing col 0 of a
  shared [128,2,32] DoubleRow stationary; colsum matmuls place m-slice k on
  out partitions 32k..32k+31 via tile_position=(0,32k), so no per-slice
  stationary copies are needed.
- Colsum: fp8e5 DoubleRow, 4 compound 1024-wide matmuls per pair (K=256),
  accumulated in PSUM banks 6-7 across all pairs; rr*64 keeps e5m2 normal
  range; 1/64 and 1/N fold into the tail.
- Tail: evict colsum -> w bf16, replicate each 32-partition slice to all
  128 partitions (4 matmuls), y = sum_m w[m] x[m,:] via scalar_tensor_tensor
  with accum (DVE+Pool, 2 slices each), out = WvT^T y + 0.5 bv in fp32, then
  the 32x32 block transpose so the result leaves as 4 x 128B DMA packets.

Numerics: rel err ~1.2e-2 vs the 2e-2 gate (numpy-sim validated).
Sharding: core c handles batch b=c//2, row half h=c%2; host sums the two
per-core partial outputs per batch.
"""

import sys

sys.path.insert(0, "/opt/trn_rl_repo")

import ml_dtypes
import numpy as np

import concourse.bass as bass
import concourse.mybir as mybir
import concourse.tile as tile
from concourse import bacc

D = 128
N = 4096
B = 4
NCORES = 8
HALF = N // 2
RT = HALF // 128  # 16 row tiles per core
GW = 1024

C_SHIFT = 13.75
A8 = 4.0 / np.log(2.0)
B8 = 60.5 - C_SHIFT * A8
SIGMA = 64.0

F32 = mybir.dt.float32
BF16 = mybir.dt.bfloat16
FP8 = mybir.dt.float8e5
NPBF = ml_dtypes.bfloat16
AF = mybir.ActivationFunctionType
ALU = mybir.AluOpType
DRM = mybir.MatmulPerfMode.DoubleRow


def build_nc():
    nc = bacc.Bacc()
    xt = nc.dram_tensor("xt", [D, N], BF16, kind="ExternalInput")  # x[b].T
    xq = nc.dram_tensor("xq", [D, HALF], BF16, kind="ExternalInput")  # row half
    wqT = nc.dram_tensor("wqT", [D, D], BF16, kind="ExternalInput")  # A8*Wq.T
    wkT = nc.dram_tensor("wkT", [D, D], BF16, kind="ExternalInput")
    wvT = nc.dram_tensor("wvT", [D, D], F32, kind="ExternalInput")
    bq = nc.dram_tensor("bq", [D, 1], F32, kind="ExternalInput")  # A8*bq
    bk = nc.dram_tensor("bk", [D, 1], F32, kind="ExternalInput")
    bvh = nc.dram_tensor("bvh", [D, 1], F32, kind="ExternalInput")  # bv*0.5
    out = nc.dram_tensor("out", [4, 32], F32, kind="ExternalOutput")

    with tile.TileContext(nc) as tc:
        with (
            tc.tile_pool(name="singles", bufs=1) as singles,
            tc.tile_pool(name="ps", bufs=1, space="PSUM") as ps,
        ):
            # ---- SBUF ----
            wq_sb = singles.tile([D, D], BF16, tag="wq", name="wq_sb")
            wk_sb = singles.tile([D, D], BF16, tag="wk", name="wk_sb")
            wv_sb = singles.tile([D, D], F32, tag="wv", name="wv_sb")
            bqs = singles.tile([D, 1], F32, tag="bq", name="bqs")
            bks = singles.tile([D, 1], F32, tag="bk", name="bks")
            bvs = singles.tile([D, 1], F32, tag="bv", name="bvs")
            xt_q = [singles.tile([D, GW], BF16, tag=f"xt{q}", name=f"xt{q}")
                    for q in range(4)]
            xq_q = [singles.tile([D, GW], BF16, tag=f"xq{q}", name=f"xq{q}")
                    for q in range(2)]
            kt_q = [singles.tile([D, GW], BF16, tag=f"kt{q}", name=f"kt{q}")
                    for q in range(4)]
            qt_q = [singles.tile([D, GW], BF16, tag=f"qt{q}", name=f"qt{q}")
                    for q in range(2)]
            E = [singles.tile([128, 2, N], FP8, tag=f"E{b_}", name=f"E{b_}")
                 for b_ in range(2)]
            # DR stationary per pair: 4 windows of 160 cols; window k =
            # [160k, 160k+128), rr*SIGMA on its local col 32k (flat 192k)
            rmat = [singles.tile([128, 2, 640], FP8, tag=f"rm{b_}",
                                 name=f"rm{b_}") for b_ in range(2)]
            zero4 = singles.tile([128, 4], F32, tag="z4", name="zero4")
            part = singles.tile([128, RT, 4], F32, tag="part", name="part")
            R2 = singles.tile([128, 2], F32, tag="R2", name="R2")
            RR2 = singles.tile([128, 2], F32, tag="RR2", name="RR2")
            cshift = singles.tile([128, 1], F32, tag="csh", name="cshift")
            ones_sb = singles.tile([D, D], BF16, tag="ones", name="ones_sb")
            wbb = singles.tile([128, GW], BF16, tag="wbb", name="wbb")
            ysc = singles.tile([128, GW], F32, tag="ysc", name="ysc")
            ycell = singles.tile([128, 4], F32, tag="ycell", name="ycell")
            yv = singles.tile([128, 1], F32, tag="yv", name="yv")
            t32 = singles.tile([128, 32], F32, tag="t32", name="t32")
            t32t = singles.tile([128, 32], F32, tag="t32t", name="t32t")
            rdump = singles.tile([128, GW], FP8, tag="rdump", name="rdump")

            nc.vector.memset(part, 0.0)
            nc.vector.memset(cshift, -C_SHIFT)
            nc.vector.memset(t32, 0.0)
            nc.vector.memset(ones_sb, 1.0)
            nc.vector.memset(zero4, 0.0)
            nc.gpsimd.memset(rmat[0], 0.0)
            nc.gpsimd.memset(rmat[1], 0.0)

            # ---- PSUM ----
            ring = ps.tile([128, 3, GW], F32, tag="ring", name="ring")
            csum = ps.tile([128, GW], F32, tag="csum", name="csum")

            # ---- DMA prologue (3 queues) ----
            nc.sync.dma_start(wk_sb, wkT[:, :])
            nc.scalar.dma_start(wq_sb, wqT[:, :])
            nc.gpsimd.dma_start(bks, bk[:, :])
            nc.sync.dma_start(xt_q[0], xt[:, 0:GW])
            nc.scalar.dma_start(bqs, bq[:, :])
            nc.gpsimd.dma_start(xt_q[1], xt[:, GW:2 * GW])
            nc.sync.dma_start(xt_q[2], xt[:, 2 * GW:3 * GW])
            nc.scalar.dma_start(xq_q[0], xq[:, 0:GW])
            nc.gpsimd.dma_start(xt_q[3], xt[:, 3 * GW:4 * GW])
            nc.sync.dma_start(xq_q[1], xq[:, GW:2 * GW])
            nc.scalar.dma_start(wv_sb, wvT[:, :])
            nc.gpsimd.dma_start(bvs, bvh[:, :])

            # ---- projections: K all 4 quarters, Q both halves ----
            # slots: Kq0->s0(ACT evict), Kq1->s1(DVE), Kq2->s2(ACT),
            #        Kq3->s0(DVE), Qq0->s1(ACT), Qq1->s2(DVE)
            def proj(dst, w_sb, src_sb, slot, bias_sb, eng):
                pt = ring[:, slot, :]
                for h2 in range(2):
                    nc.tensor.matmul(
                        pt[:, h2 * 512:(h2 + 1) * 512], w_sb,
                        src_sb[:, h2 * 512:(h2 + 1) * 512],
                        start=True, stop=True,
                    )
                if eng == "act":
                    nc.scalar.activation(
                        out=dst, in_=pt, func=AF.Identity, bias=bias_sb
                    )
                else:
                    nc.vector.tensor_scalar(
                        out=dst, in0=pt, scalar1=bias_sb, scalar2=None,
                        op0=ALU.add,
                    )

            proj(kt_q[0], wk_sb, xt_q[0], 0, bks, "act")
            proj(kt_q[1], wk_sb, xt_q[1], 1, bks, "dve")
            proj(kt_q[2], wk_sb, xt_q[2], 2, bks, "act")
            proj(kt_q[3], wk_sb, xt_q[3], 0, bks, "dve")
            proj(qt_q[0], wq_sb, xq_q[0], 1, bqs, "act")
            proj(qt_q[1], wq_sb, xq_q[1], 2, bqs, "dve")

            # ---- main loop ----
            def s_mm(slot, lhsT, rhs_q):
                # matmul out is ISA-capped at 512 (one PSUM bank)
                for h2 in range(2):
                    nc.tensor.matmul(
                        ring[:, slot, h2 * 512:(h2 + 1) * 512],
                        lhsT,
                        rhs_q[:, h2 * 512:(h2 + 1) * 512],
                        start=True, stop=True,
                    )

            def emit_colsum_k(p, k):
                # matmul rhs free size is ISA-capped at 1024 -> 2x512 halves
                for h2 in range(2):
                    nc.tensor.matmul(
                        csum[:, h2 * 512:(h2 + 1) * 512],
                        rmat[p % 2][:, :, 160 * k:160 * k + 128],
                        E[p % 2][:, :, k * GW + h2 * 512:k * GW + (h2 + 1) * 512],
                        start=(p == 0 and k == 0),
                        stop=(p == RT // 2 - 1 and k == 3),
                        perf_mode=DRM,
                        skip_group_check=True,
                    )

            for i in range(RT):
                r = i % 2  # slot rotation: even tiles (0,1,2,0), odd (1,2,0,1)
                sA, sB, sC = r, r + 1, (r + 2) % 3
                hh = i % 2
                Eb = E[(i // 2) % 2]
                lhsT = qt_q[i // 8][:, (i % 8) * 128:(i % 8) * 128 + 128]

                # S groups 0-2
                s_mm(sA, lhsT, kt_q[0])
                s_mm(sB, lhsT, kt_q[1])
                if r == 0:
                    # slots 0,1,2 ascending: one 3072-wide exp after g2 lands
                    s_mm(sC, lhsT, kt_q[2])
                    nc.scalar.activation(
                        out=Eb[:, hh, 0:3 * GW],
                        in_=ring[:, 0:3, :],
                        func=AF.Exp,
                        bias=cshift,
                        scale=1.0 / A8,
                        accum_out=part[:, i, 0:1],
                    )
                else:
                    # slots 1,2 then 0: 2048-wide pair + 1024 single
                    nc.scalar.activation(
                        out=Eb[:, hh, 0:2 * GW],
                        in_=ring[:, sA:sA + 2, :],
                        func=AF.Exp,
                        bias=cshift,
                        scale=1.0 / A8,
                        accum_out=part[:, i, 0:1],
                    )
                    s_mm(sC, lhsT, kt_q[2])
                    nc.scalar.activation(
                        out=Eb[:, hh, 2 * GW:3 * GW],
                        in_=ring[:, sC, :],
                        func=AF.Exp,
                        bias=cshift,
                        scale=1.0 / A8,
                        accum_out=part[:, i, 1:2],
                    )
                # colsum of pair p=(i-2)//2 interleaves here (2 slices/tile)
                if i >= 2:
                    p = (i - 2) // 2
                    if i % 2 == 0:
                        emit_colsum_k(p, 0)
                        emit_colsum_k(p, 1)
                    else:
                        emit_colsum_k(p, 2)
                        emit_colsum_k(p, 3)
                # S group 3 (waits on ACT freeing sA)
                s_mm(sA, lhsT, kt_q[3])
                # DVE: schraudolph bits group 3
                nc.vector.tensor_scalar(
                    out=Eb[:, hh, 3 * GW:4 * GW].bitcast(mybir.dt.int8),
                    in0=ring[:, sA, :],
                    scalar1=B8, scalar2=0.0,
                    op0=ALU.add, op1=ALU.max,
                )
                # DVE: row-sum re-read of group 3
                nc.vector.tensor_scalar(
                    out=rdump,
                    in0=Eb[:, hh, 3 * GW:4 * GW],
                    scalar1=0.0, scalar2=0.0,
                    op0=ALU.add, op1=ALU.add,
                    accum_out=part[:, i, 2:3],
                )
                # rr chain per pair
                if i % 2 == 1:
                    p = i // 2
                    nc.vector.tensor_reduce(
                        out=R2, in_=part[:, 2 * p:2 * p + 2, :],
                        axis=mybir.AxisListType.X, op=ALU.add,
                    )
                    nc.vector.reciprocal(out=RR2, in_=R2)
                    for h2 in range(2):
                        nc.vector.tensor_scalar(
                            out=rmat[p % 2][:, h2, 0:577:192],
                            in0=zero4,
                            scalar1=RR2[:, h2:h2 + 1], scalar2=SIGMA,
                            op0=ALU.add, op1=ALU.mult,
                        )

            # last pair's colsum
            for k in range(4):
                emit_colsum_k(RT // 2 - 1, k)

            # ---- tail ----
            # w (x SIGMA*N) -> bf16
            nc.scalar.activation(out=wbb, in_=csum, func=AF.Identity)
            # replicate each 32-partition slice to all 128 partitions, then
            # y = sum_m w[m] * x[m, :]  (DVE; Pool cannot read PSUM)
            for k in range(4):
                wrep = ring[:, k % 3, :]
                for h2 in range(2):
                    nc.tensor.matmul(
                        wrep[:, h2 * 512:(h2 + 1) * 512],
                        ones_sb[32 * k:32 * k + 1, :],
                        wbb[32 * k:32 * k + 1, h2 * 512:(h2 + 1) * 512],
                        start=True, stop=True,
                        tile_position=(32 * k, 0),
                    )
                nc.vector.scalar_tensor_tensor(
                    out=ysc,
                    in0=xt_q[k], scalar=1.0, in1=wrep,
                    op0=ALU.mult, op1=ALU.mult,
                    accum_out=ycell[:, k:k + 1],
                )
            nc.vector.tensor_reduce(
                out=yv, in_=ycell, axis=mybir.AxisListType.X, op=ALU.add
            )
            # scale by 1/(N*SIGMA) during f32 keep
            nc.vector.tensor_scalar(
                out=yv, in0=yv, scalar1=1.0 / (N * SIGMA), scalar2=None,
                op0=ALU.mult,
            )
            # out[d] = sum_e WvT[e, d] * y[e] + 0.5*bv  (fp32 matmul)
            op = ring[:, 2, 0:1]
            nc.tensor.matmul(op, wv_sb, yv, start=True, stop=True)
            nc.scalar.activation(
                out=t32[:, 0:1], in_=op, func=AF.Identity, bias=bvs
            )
            nc.vector.transpose(out=t32t, in_=t32)
            nc.sync.dma_start(out[:, :], t32t[0:128:32, :])

    nc.compile()
    return nc


_cache = {}


def get_nc():
    if "nc" not in _cache:
        _cache["nc"] = build_nc()
    return _cache["nc"]


def make_in_maps(x, Wq, bq, Wk, bk, Wv, bv):
    x = np.asarray(x, np.float32)
    wqT = np.ascontiguousarray((A8 * np.asarray(Wq, np.float32)).T.astype(NPBF))
    wkT = np.ascontiguousarray(np.asarray(Wk, np.float32).T.astype(NPBF))
    wvT = np.ascontiguousarray(np.asarray(Wv, np.float32).T.astype(np.float32))
    bqc = np.ascontiguousarray(A8 * np.asarray(bq, np.float32).reshape(D, 1))
    bkc = np.ascontiguousarray(np.asarray(bk, np.float32).reshape(D, 1))
    bvc = np.ascontiguousarray(0.5 * np.asarray(bv, np.float32).reshape(D, 1))
    in_maps = []
    for c in range(NCORES):
        b = c // 2
        h = c % 2
        xbT = np.ascontiguousarray(x[b].T.astype(NPBF))  # [128, 4096] bf16
        in_maps.append(
            {
                "xt": xbT,
                "xq": np.ascontiguousarray(xbT[:, h * HALF:(h + 1) * HALF]),
                "wqT": wqT,
                "wkT": wkT,
                "wvT": wvT,
                "bq": bqc,
                "bk": bkc,
                "bvh": bvc,
            }
        )
    return in_maps


def combine(results):
    outs = [np.asarray(results[c]["out"]).reshape(D) for c in range(NCORES)]
    return np.stack([outs[2 * b] + outs[2 * b + 1] for b in range(B)]).astype(
        np.float32
    )


def run(inputs, trace=False, **kwargs):
    from concourse.bass_utils import run_bass_kernel_spmd

    nc = get_nc()
    in_maps = make_in_maps(**inputs)
    res = run_bass_kernel_spmd(
        nc, in_maps, core_ids=list(range(NCORES)), trace=trace, **kwargs
    )
    return combine(res.results), res


def kernel(x, Wq, bq, Wk, bk, Wv, bv):
    out, _ = run(dict(x=x, Wq=Wq, bq=bq, Wk=Wk, bk=bk, Wv=Wv, bv=bv))
    return out


# revision 15
# speedup vs baseline: 1.0049x; 1.0049x over previous
"""AttentionAggregation kernel for 8 TRN2 NeuronCores (v2 restructure).

Math: out[b] = mean_n softmax(Q K^T)[n,:] @ V  with Q/K/V = x @ W^T + b.
Fold: out[b,d] = sum_m w[b,m] V[b,m,d],  w[b,m] = (1/N) sum_n E[n,m]/R[n],
E = exp(S - c), R[n] = rowsum E.  Fixed shift c sized so E fits fp8e5 range.
V is never materialized: out = (w @ x) @ Wv^T + 0.5*bv (since sum_m w[m] = 1/2
per core), so the V projection and its eviction disappear from the main loop.

Structure per core (batch b = c//2, row half h = c%2, 16 row tiles of 128):
- PSUM: banks 0-5 = three 1024-col slots for S groups; banks 6-7 = colsum
  accumulator, live across all 8 pairs (no per-pair evictions).  Slot
  rotation alternates r in {0,1} per tile: writes (g0,g1,g2,g3) ->
  (r, r+1, r+2 mod 3, r); the ACT exp pair always sees a contiguous
  ascending 2048-col window and every slot's next writer follows the
  reader that frees it.
- Exp split: groups 0-1 on ACT (one 2048-wide exp + accum for R), groups
  2-3 Schraudolph on DVE (int8-bitcast e5m2 bits; host pre-scales Wq/bq by
  A8=4/ln2), row-sum re-read of groups 2-3 on Pool (one 2048-wide
  tensor_reduce into an R cell).  Slots only ever hold S'/projection values
  (<= ~117 in e5m2-bit units), so a stale int8 convert can never saturate
  into the e5m2 NaN code 127.
- rr = 1/R per pair: one reduce + reciprocal + fp8 cast writing col 0 of a
  shared [128,2,32] DoubleRow stationary; colsum matmuls place m-slice k on
  out partitions 32k..32k+31 via tile_position=(0,32k), so no per-slice
  stationary copies are needed.
- Colsum: fp8e5 DoubleRow, 4 compound 1024-wide matmuls per pair (K=256),
  accumulated in PSUM banks 6-7 across all pairs; rr*64 keeps e5m2 normal
  range; 1/64 and 1/N fold into the tail.
- Tail: evict colsum -> w bf16, replicate each 32-partition slice to all
  128 partitions (4 matmuls), y = sum_m w[m] x[m,:] via scalar_tensor_tensor
  with accum (DVE+Pool, 2 slices each), out = WvT^T y + 0.5 bv in fp32, then
  the 32x32 block transpose so the result leaves as 4 x 128B DMA packets.

Numerics: rel err ~1.2e-2 vs the 2e-2 gate (numpy-sim validated).
Sharding: core c handles batch b=c//2, row half h=c%2; host sums the two
per-core partial outputs per batch.
"""

import sys

sys.path.insert(0, "/opt/trn_rl_repo")

import ml_dtypes
import numpy as np

import concourse.bass as bass
import concourse.mybir as mybir
import concourse.tile as tile
from concourse import bacc

D = 128
N = 4096
B = 4
NCORES = 8
HALF = N // 2
RT = HALF // 128  # 16 row tiles per core
GW = 1024

C_SHIFT = 13.75
A8 = 4.0 / np.log(2.0)
B8 = 60.5 - C_SHIFT * A8
SIGMA = 64.0

F32 = mybir.dt.float32
BF16 = mybir.dt.bfloat16
FP8 = mybir.dt.float8e5
NPBF = ml_dtypes.bfloat16
AF = mybir.ActivationFunctionType
ALU = mybir.AluOpType
DRM = mybir.MatmulPerfMode.DoubleRow


def build_nc():
    nc = bacc.Bacc()
    xt = nc.dram_tensor("xt", [D, N], BF16, kind="ExternalInput")  # x[b].T
    xq = nc.dram_tensor("xq", [D, HALF], BF16, kind="ExternalInput")  # row half
    wqT = nc.dram_tensor("wqT", [D, D], BF16, kind="ExternalInput")  # A8*Wq.T
    wkT = nc.dram_tensor("wkT", [D, D], BF16, kind="ExternalInput")
    wvT = nc.dram_tensor("wvT", [D, D], F32, kind="ExternalInput")
    bq = nc.dram_tensor("bq", [D, 1], F32, kind="ExternalInput")  # A8*bq
    bk = nc.dram_tensor("bk", [D, 1], F32, kind="ExternalInput")
    bvh = nc.dram_tensor("bvh", [D, 1], F32, kind="ExternalInput")  # bv*0.5
    out = nc.dram_tensor("out", [4, 32], F32, kind="ExternalOutput")

    with tile.TileContext(nc) as tc:
        with (
            tc.tile_pool(name="singles", bufs=1) as singles,
            tc.tile_pool(name="ps", bufs=1, space="PSUM") as ps,
        ):
            # ---- SBUF ----
            wq_sb = singles.tile([D, D], BF16, tag="wq", name="wq_sb")
            wk_sb = singles.tile([D, D], BF16, tag="wk", name="wk_sb")
            wv_sb = singles.tile([D, D], F32, tag="wv", name="wv_sb")
            bqs = singles.tile([D, 1], F32, tag="bq", name="bqs")
            bks = singles.tile([D, 1], F32, tag="bk", name="bks")
            bvs = singles.tile([D, 1], F32, tag="bv", name="bvs")
            xt_q = [singles.tile([D, GW], BF16, tag=f"xt{q}", name=f"xt{q}")
                    for q in range(4)]
            xq_q = [singles.tile([D, GW], BF16, tag=f"xq{q}", name=f"xq{q}")
                    for q in range(2)]
            kt_q = [singles.tile([D, GW], BF16, tag=f"kt{q}", name=f"kt{q}")
                    for q in range(4)]
            qt_q = [singles.tile([D, GW], BF16, tag=f"qt{q}", name=f"qt{q}")
                    for q in range(2)]
            E = [singles.tile([128, 2, N], FP8, tag=f"E{b_}", name=f"E{b_}")
                 for b_ in range(2)]
            # DR stationary per pair: 4 windows of 160 cols; window k =
            # [160k, 160k+128), rr*SIGMA on its local col 32k (flat 192k)
            rmat = [singles.tile([128, 2, 640], FP8, tag=f"rm{b_}",
                                 name=f"rm{b_}") for b_ in range(2)]
            zero4 = singles.tile([128, 4], F32, tag="z4", name="zero4")
            part = singles.tile([128, RT, 4], F32, tag="part", name="part")
            R2 = singles.tile([128, 2], F32, tag="R2", name="R2")
            RR2 = singles.tile([128, 2], F32, tag="RR2", name="RR2")
            cshift = singles.tile([128, 1], F32, tag="csh", name="cshift")
            ones_sb = singles.tile([D, D], BF16, tag="ones", name="ones_sb")
            wbb = singles.tile([128, GW], BF16, tag="wbb", name="wbb")
            ysc = singles.tile([128, GW], F32, tag="ysc", name="ysc")
            ycell = singles.tile([128, 4], F32, tag="ycell", name="ycell")
            yv = singles.tile([128, 1], F32, tag="yv", name="yv")
            t32 = singles.tile([128, 32], F32, tag="t32", name="t32")
            t32t = singles.tile([128, 32], F32, tag="t32t", name="t32t")
            rdump = singles.tile([128, GW], FP8, tag="rdump", name="rdump")

            nc.vector.memset(part, 0.0)
            nc.vector.memset(cshift, -C_SHIFT)
            nc.vector.memset(t32, 0.0)
            nc.vector.memset(ones_sb, 1.0)
            nc.vector.memset(zero4, 0.0)
            nc.gpsimd.memset(rmat[0], 0.0)
            nc.gpsimd.memset(rmat[1], 0.0)

            # ---- PSUM ----
            ring = ps.tile([128, 3, GW], F32, tag="ring", name="ring")
            csum = ps.tile([128, GW], F32, tag="csum", name="csum")

            # ---- DMA prologue (3 queues) ----
            nc.sync.dma_start(wk_sb, wkT[:, :])
            nc.scalar.dma_start(wq_sb, wqT[:, :])
            nc.gpsimd.dma_start(bks, bk[:, :])
            nc.sync.dma_start(xt_q[0], xt[:, 0:GW])
            nc.scalar.dma_start(bqs, bq[:, :])
            nc.gpsimd.dma_start(xt_q[1], xt[:, GW:2 * GW])
            nc.sync.dma_start(xt_q[2], xt[:, 2 * GW:3 * GW])
            nc.scalar.dma_start(xq_q[0], xq[:, 0:GW])
            nc.gpsimd.dma_start(xt_q[3], xt[:, 3 * GW:4 * GW])
            nc.sync.dma_start(xq_q[1], xq[:, GW:2 * GW])
            nc.scalar.dma_start(wv_sb, wvT[:, :])
            nc.gpsimd.dma_start(bvs, bvh[:, :])

            # ---- projections: K all 4 quarters, Q both halves ----
            # 512-granular: each half evicted (ACT+DVE in parallel) as soon
            # as its matmul lands, so projections pipeline at ~1.2us each
            def proj(dst, w_sb, src_sb, slot, bias_sb):
                pt = ring[:, slot, :]
                for h2 in range(2):
                    nc.tensor.matmul(
                        pt[:, h2 * 512:(h2 + 1) * 512], w_sb,
                        src_sb[:, h2 * 512:(h2 + 1) * 512],
                        start=True, stop=True,
                    )
                    if h2 == 0:
                        nc.scalar.activation(
                            out=dst[:, 0:512], in_=pt[:, 0:512],
                            func=AF.Identity, bias=bias_sb,
                        )
                    else:
                        nc.vector.tensor_scalar(
                            out=dst[:, 512:1024], in0=pt[:, 512:1024],
                            scalar1=bias_sb, scalar2=None, op0=ALU.add,
                        )

            proj(kt_q[0], wk_sb, xt_q[0], 0, bks)
            proj(qt_q[0], wq_sb, xq_q[0], 1, bqs)
            proj(kt_q[1], wk_sb, xt_q[1], 2, bks)
            proj(kt_q[2], wk_sb, xt_q[2], 0, bks)
            proj(kt_q[3], wk_sb, xt_q[3], 1, bqs if False else bks)
            proj(qt_q[1], wq_sb, xq_q[1], 2, bqs)

            # ---- main loop ----
            def s_mm(slot, lhsT, rhs_q):
                # matmul out is ISA-capped at 512 (one PSUM bank)
                for h2 in range(2):
                    nc.tensor.matmul(
                        ring[:, slot, h2 * 512:(h2 + 1) * 512],
                        lhsT,
                        rhs_q[:, h2 * 512:(h2 + 1) * 512],
                        start=True, stop=True,
                    )

            def emit_colsum_k(p, k):
                # matmul rhs free size is ISA-capped at 1024 -> 2x512 halves
                for h2 in range(2):
                    nc.tensor.matmul(
                        csum[:, h2 * 512:(h2 + 1) * 512],
                        rmat[p % 2][:, :, 160 * k:160 * k + 128],
                        E[p % 2][:, :, k * GW + h2 * 512:k * GW + (h2 + 1) * 512],
                        start=(p == 0 and k == 0),
                        stop=(p == RT // 2 - 1 and k == 3),
                        perf_mode=DRM,
                        skip_group_check=True,
                    )

            for i in range(RT):
                # fixed slots: s0,s1 feed the ACT 2048 pair exp; s2 is used
                # twice (DVE schraudolph group, then ACT 1024 group)
                hh = i % 2
                Eb = E[(i // 2) % 2]
                lhsT = qt_q[i // 8][:, (i % 8) * 128:(i % 8) * 128 + 128]

                s_mm(0, lhsT, kt_q[0])
                s_mm(1, lhsT, kt_q[1])
                s_mm(2, lhsT, kt_q[2])
                # ACT: pair exp (banks 0-3, always contiguous)
                nc.scalar.activation(
                    out=Eb[:, hh, 0:2 * GW],
                    in_=ring[:, 0:2, :],
                    func=AF.Exp,
                    bias=cshift,
                    scale=1.0 / A8,
                    accum_out=part[:, i, 0:1],
                )
                # DVE: schraudolph bits for cols 2048-3072 (slot 2)
                nc.vector.tensor_scalar(
                    out=Eb[:, hh, 2 * GW:3 * GW].bitcast(mybir.dt.int8),
                    in0=ring[:, 2, :],
                    scalar1=B8, scalar2=0.0,
                    op0=ALU.add, op1=ALU.max,
                )
                # colsum of pair p=(i-2)//2 interleaves here (2 slices/tile)
                if i >= 2:
                    p = (i - 2) // 2
                    if i % 2 == 0:
                        emit_colsum_k(p, 0)
                        emit_colsum_k(p, 1)
                    else:
                        emit_colsum_k(p, 2)
                        emit_colsum_k(p, 3)
                # S group 3 reuses slot 2 once the bits pass freed it
                s_mm(2, lhsT, kt_q[3])
                # DVE: row-sum re-read of the schraudolph group
                nc.vector.tensor_scalar(
                    out=rdump,
                    in0=Eb[:, hh, 2 * GW:3 * GW],
                    scalar1=0.0, scalar2=0.0,
                    op0=ALU.add, op1=ALU.add,
                    accum_out=part[:, i, 2:3],
                )
                # ACT: exp of cols 3072-4096 (slot 2 again)
                nc.scalar.activation(
                    out=Eb[:, hh, 3 * GW:4 * GW],
                    in_=ring[:, 2, :],
                    func=AF.Exp,
                    bias=cshift,
                    scale=1.0 / A8,
                    accum_out=part[:, i, 1:2],
                )
                # rr chain per pair
                if i % 2 == 1:
                    p = i // 2
                    nc.vector.tensor_reduce(
                        out=R2, in_=part[:, 2 * p:2 * p + 2, :],
                        axis=mybir.AxisListType.X, op=ALU.add,
                    )
                    nc.vector.reciprocal(out=RR2, in_=R2)
                    for h2 in range(2):
                        nc.vector.tensor_scalar(
                            out=rmat[p % 2][:, h2, 0:577:192],
                            in0=zero4,
                            scalar1=RR2[:, h2:h2 + 1], scalar2=SIGMA,
                            op0=ALU.add, op1=ALU.mult,
                        )

            # last pair's colsum
            for k in range(4):
                emit_colsum_k(RT // 2 - 1, k)

            # ---- tail ----
            # w (x SIGMA*N) -> bf16
            nc.scalar.activation(out=wbb, in_=csum, func=AF.Identity)
            # replicate each 32-partition slice to all 128 partitions into
            # 4 distinct psum regions (slots 0-2 + freed csum banks), then
            # the 4 y-accumulations run back-to-back on DVE with no stalls
            wrep_t = [ring[:, 0, :], ring[:, 1, :], ring[:, 2, :], csum]
            for k in range(4):
                for h2 in range(2):
                    nc.tensor.matmul(
                        wrep_t[k][:, h2 * 512:(h2 + 1) * 512],
                        ones_sb[32 * k:32 * k + 1, :],
                        wbb[32 * k:32 * k + 1, h2 * 512:(h2 + 1) * 512],
                        start=True, stop=True,
                        tile_position=(32 * k, 0),
                    )
            for k in range(4):
                nc.vector.scalar_tensor_tensor(
                    out=ysc,
                    in0=xt_q[k], scalar=1.0, in1=wrep_t[k],
                    op0=ALU.mult, op1=ALU.mult,
                    accum_out=ycell[:, k:k + 1],
                )
            nc.vector.tensor_reduce(
                out=yv, in_=ycell, axis=mybir.AxisListType.X, op=ALU.add
            )
            # scale by 1/(N*SIGMA) during f32 keep
            nc.vector.tensor_scalar(
                out=yv, in0=yv, scalar1=1.0 / (N * SIGMA), scalar2=None,
                op0=ALU.mult,
            )
            # out[d] = sum_e WvT[e, d] * y[e] + 0.5*bv  (fp32 matmul)
            op = ring[:, 2, 0:1]
            nc.tensor.matmul(op, wv_sb, yv, start=True, stop=True)
            nc.scalar.activation(
                out=t32[:, 0:1], in_=op, func=AF.Identity, bias=bvs
            )
            nc.vector.transpose(out=t32t, in_=t32)
            nc.sync.dma_start(out[:, :], t32t[0:128:32, :])

    nc.compile()
    return nc


_cache = {}


def get_nc():
    if "nc" not in _cache:
        _cache["nc"] = build_nc()
    return _cache["nc"]


def make_in_maps(x, Wq, bq, Wk, bk, Wv, bv):
    x = np.asarray(x, np.float32)
    wqT = np.ascontiguousarray((A8 * np.asarray(Wq, np.float32)).T.astype(NPBF))
    wkT = np.ascontiguousarray(np.asarray(Wk, np.float32).T.astype(NPBF))
    wvT = np.ascontiguousarray(np.asarray(Wv, np.float32).T.astype(np.float32))
    bqc = np.ascontiguousarray(A8 * np.asarray(bq, np.float32).reshape(D, 1))
    bkc = np.ascontiguousarray(np.asarray(bk, np.float32).reshape(D, 1))
    bvc = np.ascontiguousarray(0.5 * np.asarray(bv, np.float32).reshape(D, 1))
    in_maps = []
    for c in range(NCORES):
        b = c // 2
        h = c % 2
        xbT = np.ascontiguousarray(x[b].T.astype(NPBF))  # [128, 4096] bf16
        in_maps.append(
            {
                "xt": xbT,
                "xq": np.ascontiguousarray(xbT[:, h * HALF:(h + 1) * HALF]),
                "wqT": wqT,
                "wkT": wkT,
                "wvT": wvT,
                "bq": bqc,
                "bk": bkc,
                "bvh": bvc,
            }
        )
    return in_maps


def combine(results):
    outs = [np.asarray(results[c]["out"]).reshape(D) for c in range(NCORES)]
    return np.stack([outs[2 * b] + outs[2 * b + 1] for b in range(B)]).astype(
        np.float32
    )


def run(inputs, trace=False, **kwargs):
    from concourse.bass_utils import run_bass_kernel_spmd

    nc = get_nc()
    in_maps = make_in_maps(**inputs)
    res = run_bass_kernel_spmd(
        nc, in_maps, core_ids=list(range(NCORES)), trace=trace, **kwargs
    )
    return combine(res.results), res


def kernel(x, Wq, bq, Wk, bk, Wv, bv):
    out, _ = run(dict(x=x, Wq=Wq, bq=bq, Wk=Wk, bk=bk, Wv=Wv, bv=bv))
    return out


# revision 17
# speedup vs baseline: 1.5346x; 1.5271x over previous
"""AttentionAggregation kernel for 8 TRN2 NeuronCores (v2 restructure).

Math: out[b] = mean_n softmax(Q K^T)[n,:] @ V  with Q/K/V = x @ W^T + b.
Fold: out[b,d] = sum_m w[b,m] V[b,m,d],  w[b,m] = (1/N) sum_n E[n,m]/R[n],
E = exp(S - c), R[n] = rowsum E.  Fixed shift c sized so E fits fp8e5 range.
V is never materialized: out = (w @ x) @ Wv^T + 0.5*bv (since sum_m w[m] = 1/2
per core), so the V projection and its eviction disappear from the main loop.

Structure per core (batch b = c//2, row half h = c%2, 16 row tiles of 128):
- PSUM: banks 0-5 = three 1024-col slots for S groups; banks 6-7 = colsum
  accumulator, live across all 8 pairs (no per-pair evictions).  Slot
  rotation alternates r in {0,1} per tile: writes (g0,g1,g2,g3) ->
  (r, r+1, r+2 mod 3, r); the ACT exp pair always sees a contiguous
  ascending 2048-col window and every slot's next writer follows the
  reader that frees it.
- Exp split: groups 0-1 on ACT (one 2048-wide exp + accum for R), groups
  2-3 Schraudolph on DVE (int8-bitcast e5m2 bits; host pre-scales Wq/bq by
  A8=4/ln2), row-sum re-read of groups 2-3 on Pool (one 2048-wide
  tensor_reduce into an R cell).  Slots only ever hold S'/projection values
  (<= ~117 in e5m2-bit units), so a stale int8 convert can never saturate
  into the e5m2 NaN code 127.
- rr = 1/R per pair: one reduce + reciprocal + fp8 cast writing col 0 of a
  shared [128,2,32] DoubleRow stationary; colsum matmuls place m-slice k on
  out partitions 32k..32k+31 via tile_position=(0,32k), so no per-slice
  stationary copies are needed.
- Colsum: fp8e5 DoubleRow, 4 compound 1024-wide matmuls per pair (K=256),
  accumulated in PSUM banks 6-7 across all pairs; rr*64 keeps e5m2 normal
  range; 1/64 and 1/N fold into the tail.
- Tail: evict colsum -> w bf16, replicate each 32-partition slice to all
  128 partitions (4 matmuls), y = sum_m w[m] x[m,:] via scalar_tensor_tensor
  with accum (DVE+Pool, 2 slices each), out = WvT^T y + 0.5 bv in fp32, then
  the 32x32 block transpose so the result leaves as 4 x 128B DMA packets.

Numerics: rel err ~1.2e-2 vs the 2e-2 gate (numpy-sim validated).
Sharding: core c handles batch b=c//2, row half h=c%2; host sums the two
per-core partial outputs per batch.
"""

import sys

sys.path.insert(0, "/opt/trn_rl_repo")

import ml_dtypes
import numpy as np

import concourse.bass as bass
import concourse.mybir as mybir
import concourse.tile as tile
from concourse import bacc

D = 128
N = 4096
B = 4
NCORES = 8
HALF = N // 2
RT = HALF // 128  # 16 row tiles per core
GW = 1024

C_SHIFT = 13.75
A8 = 4.0 / np.log(2.0)
B8 = 60.5 - C_SHIFT * A8
SIGMA = 64.0

F32 = mybir.dt.float32
BF16 = mybir.dt.bfloat16
FP8 = mybir.dt.float8e5
NPBF = ml_dtypes.bfloat16
AF = mybir.ActivationFunctionType
ALU = mybir.AluOpType
DRM = mybir.MatmulPerfMode.DoubleRow


def build_nc():
    nc = bacc.Bacc()
    xt = nc.dram_tensor("xt", [D, N], BF16, kind="ExternalInput")  # x[b].T
    xq = nc.dram_tensor("xq", [D, HALF], BF16, kind="ExternalInput")  # row half
    wqT = nc.dram_tensor("wqT", [D, D], BF16, kind="ExternalInput")  # A8*Wq.T
    wkT = nc.dram_tensor("wkT", [D, D], BF16, kind="ExternalInput")
    wvT = nc.dram_tensor("wvT", [D, D], F32, kind="ExternalInput")
    bq = nc.dram_tensor("bq", [D, 1], F32, kind="ExternalInput")  # A8*bq
    bk = nc.dram_tensor("bk", [D, 1], F32, kind="ExternalInput")
    bvh = nc.dram_tensor("bvh", [D, 1], F32, kind="ExternalInput")  # bv*0.5
    out = nc.dram_tensor("out", [4, 32], F32, kind="ExternalOutput")

    with tile.TileContext(nc) as tc:
        with (
            tc.tile_pool(name="singles", bufs=1) as singles,
            tc.tile_pool(name="ps", bufs=1, space="PSUM") as ps,
        ):
            # ---- SBUF ----
            wq_sb = singles.tile([D, D], BF16, tag="wq", name="wq_sb")
            wk_sb = singles.tile([D, D], BF16, tag="wk", name="wk_sb")
            wv_sb = singles.tile([D, D], F32, tag="wv", name="wv_sb")
            bqs = singles.tile([D, 1], F32, tag="bq", name="bqs")
            bks = singles.tile([D, 1], F32, tag="bk", name="bks")
            bvs = singles.tile([D, 1], F32, tag="bv", name="bvs")
            xt_q = [singles.tile([D, GW], BF16, tag=f"xt{q}", name=f"xt{q}")
                    for q in range(4)]
            xq_q = [singles.tile([D, GW], BF16, tag=f"xq{q}", name=f"xq{q}")
                    for q in range(2)]
            # K/Q in 512-col half tiles so ACT/DVE evictions and S matmuls
            # track dependencies per half (no write-write false ordering)
            kt_h = [singles.tile([D, 512], BF16, tag=f"kt{h}", name=f"kt{h}")
                    for h in range(8)]
            qt_h = [singles.tile([D, 512], BF16, tag=f"qt{h}", name=f"qt{h}")
                    for h in range(4)]
            # E split per writer: ACT pair (cols 0-2048), DVE schraudolph
            # (2048-3072), ACT single (3072-4096) -- separate tiles so the
            # three writers never order against each other
            E_a = [singles.tile([128, 2, 2 * GW], FP8, tag=f"Ea{b_}",
                                name=f"Ea{b_}") for b_ in range(2)]
            E_d = [singles.tile([128, 2, GW], FP8, tag=f"Ed{b_}",
                                name=f"Ed{b_}") for b_ in range(2)]
            E_b = [singles.tile([128, 2, GW], FP8, tag=f"Eb{b_}",
                                name=f"Eb{b_}") for b_ in range(2)]
            # DR stationary per pair: 4 windows of 160 cols; window k =
            # [160k, 160k+128), rr*SIGMA on its local col 32k (flat 192k)
            rmat = [singles.tile([128, 2, 640], FP8, tag=f"rm{b_}",
                                 name=f"rm{b_}") for b_ in range(2)]
            zero4 = singles.tile([128, 4], F32, tag="z4", name="zero4")
            part_a = singles.tile([128, RT, 2], F32, tag="parta", name="parta")
            part_d = singles.tile([128, RT], F32, tag="partd", name="partd")
            R2 = singles.tile([128, 2], F32, tag="R2", name="R2")
            RR2 = singles.tile([128, 2], F32, tag="RR2", name="RR2")
            cshift = singles.tile([128, 1], F32, tag="csh", name="cshift")
            ones_sb = singles.tile([D, D], BF16, tag="ones", name="ones_sb")
            wbb = singles.tile([128, GW], BF16, tag="wbb", name="wbb")
            ysc = singles.tile([128, GW], F32, tag="ysc", name="ysc")
            ycell = singles.tile([128, 4], F32, tag="ycell", name="ycell")
            yv = singles.tile([128, 1], F32, tag="yv", name="yv")
            t32 = singles.tile([128, 32], F32, tag="t32", name="t32")
            t32t = singles.tile([128, 32], F32, tag="t32t", name="t32t")
            rdump = singles.tile([128, GW], FP8, tag="rdump", name="rdump")

            nc.vector.memset(part_a, 0.0)
            nc.vector.memset(part_d, 0.0)
            nc.vector.memset(cshift, -C_SHIFT)
            nc.vector.memset(t32, 0.0)
            nc.vector.memset(ones_sb, 1.0)
            nc.vector.memset(zero4, 0.0)
            nc.gpsimd.memset(rmat[0], 0.0)
            nc.gpsimd.memset(rmat[1], 0.0)

            # ---- PSUM ----
            # slots 0,1 share one 4-bank tile (the ACT pair exp reads the
            # whole thing, so whole-tile WAR tracking is exact); slot 2 and
            # the colsum accumulator are their own tiles
            ringA = ps.tile([128, 2, GW], F32, tag="ringA", name="ringA")
            ring2 = ps.tile([128, GW], F32, tag="ring2", name="ring2")
            csum = ps.tile([128, GW], F32, tag="csum", name="csum")

            # ---- DMA prologue (3 queues) ----
            nc.sync.dma_start(wk_sb, wkT[:, :])
            nc.scalar.dma_start(wq_sb, wqT[:, :])
            nc.gpsimd.dma_start(bks, bk[:, :])
            nc.sync.dma_start(xt_q[0], xt[:, 0:GW])
            nc.scalar.dma_start(bqs, bq[:, :])
            nc.gpsimd.dma_start(xt_q[1], xt[:, GW:2 * GW])
            nc.sync.dma_start(xt_q[2], xt[:, 2 * GW:3 * GW])
            nc.scalar.dma_start(xq_q[0], xq[:, 0:GW])
            nc.gpsimd.dma_start(xt_q[3], xt[:, 3 * GW:4 * GW])
            nc.sync.dma_start(xq_q[1], xq[:, GW:2 * GW])
            nc.scalar.dma_start(wv_sb, wvT[:, :])
            nc.gpsimd.dma_start(bvs, bvh[:, :])

            # ---- projections: K all 4 quarters, Q both halves ----
            # 512-granular: each half is its own SBUF tile, evicted on
            # ACT/DVE in parallel the moment its matmul lands
            def slot_ap(slot):
                return ringA[:, slot, :] if slot < 2 else ring2

            def proj(dsts, w_sb, src_sb, slot, bias_sb):
                pt = slot_ap(slot)
                for h2 in range(2):
                    nc.tensor.matmul(
                        pt[:, h2 * 512:(h2 + 1) * 512], w_sb,
                        src_sb[:, h2 * 512:(h2 + 1) * 512],
                        start=True, stop=True,
                    )
                    if h2 == 0:
                        nc.scalar.activation(
                            out=dsts[0], in_=pt[:, 0:512],
                            func=AF.Identity, bias=bias_sb,
                        )
                    else:
                        nc.vector.tensor_scalar(
                            out=dsts[1], in0=pt[:, 512:1024],
                            scalar1=bias_sb, scalar2=None, op0=ALU.add,
                        )

            proj(kt_h[0:2], wk_sb, xt_q[0], 0, bks)
            proj(qt_h[0:2], wq_sb, xq_q[0], 1, bqs)
            proj(kt_h[2:4], wk_sb, xt_q[1], 2, bks)
            proj(kt_h[4:6], wk_sb, xt_q[2], 0, bks)
            proj(kt_h[6:8], wk_sb, xt_q[3], 1, bks)
            proj(qt_h[2:4], wq_sb, xq_q[1], 2, bqs)

            # ---- main loop ----
            def s_mm(slot, lhsT, q):
                # matmul out is ISA-capped at 512 (one PSUM bank)
                pt = slot_ap(slot)
                for h2 in range(2):
                    nc.tensor.matmul(
                        pt[:, h2 * 512:(h2 + 1) * 512],
                        lhsT,
                        kt_h[2 * q + h2],
                        start=True, stop=True,
                    )

            def emit_colsum_k(p, k):
                # matmul rhs free size is ISA-capped at 1024 -> 2x512 halves
                if k < 2:
                    esrc = lambda h2: E_a[p % 2][:, :, k * GW + h2 * 512:
                                                 k * GW + (h2 + 1) * 512]
                elif k == 2:
                    esrc = lambda h2: E_d[p % 2][:, :, h2 * 512:(h2 + 1) * 512]
                else:
                    esrc = lambda h2: E_b[p % 2][:, :, h2 * 512:(h2 + 1) * 512]
                for h2 in range(2):
                    nc.tensor.matmul(
                        csum[:, h2 * 512:(h2 + 1) * 512],
                        rmat[p % 2][:, :, 160 * k:160 * k + 128],
                        esrc(h2),
                        start=(p == 0 and k == 0),
                        stop=(p == RT // 2 - 1 and k == 3),
                        perf_mode=DRM,
                        skip_group_check=True,
                    )

            for i in range(RT):
                # fixed slots: s0,s1 feed the ACT 2048 pair exp; s2 is used
                # twice (DVE schraudolph group, then ACT 1024 group)
                hh = i % 2
                bb = (i // 2) % 2
                lhsT = qt_h[i // 4][:, (i % 4) * 128:(i % 4) * 128 + 128]

                s_mm(0, lhsT, 0)
                s_mm(1, lhsT, 1)
                s_mm(2, lhsT, 2)
                # ACT: pair exp over slots 0,1 (banks 0-3, contiguous)
                nc.scalar.activation(
                    out=E_a[bb][:, hh, :],
                    in_=ringA[:, :, :],
                    func=AF.Exp,
                    bias=cshift,
                    scale=1.0 / A8,
                    accum_out=part_a[:, i, 0:1],
                )
                # DVE: schraudolph bits for cols 2048-3072 (slot 2)
                nc.vector.tensor_scalar(
                    out=E_d[bb][:, hh, :].bitcast(mybir.dt.int8),
                    in0=ring2,
                    scalar1=B8, scalar2=0.0,
                    op0=ALU.add, op1=ALU.max,
                )
                # colsum of pair p=(i-2)//2 interleaves here (2 slices/tile)
                if i >= 2:
                    p = (i - 2) // 2
                    if i % 2 == 0:
                        emit_colsum_k(p, 0)
                        emit_colsum_k(p, 1)
                    else:
                        emit_colsum_k(p, 2)
                        emit_colsum_k(p, 3)
                # S group 3 reuses slot 2 once the bits pass freed it
                s_mm(2, lhsT, 3)
                # DVE: row-sum re-read of the schraudolph group
                nc.vector.tensor_scalar(
                    out=rdump,
                    in0=E_d[bb][:, hh, :],
                    scalar1=0.0, scalar2=0.0,
                    op0=ALU.add, op1=ALU.add,
                    accum_out=part_d[:, i:i + 1],
                )
                # ACT: exp of cols 3072-4096 (slot 2 again)
                nc.scalar.activation(
                    out=E_b[bb][:, hh, :],
                    in_=ring2,
                    func=AF.Exp,
                    bias=cshift,
                    scale=1.0 / A8,
                    accum_out=part_a[:, i, 1:2],
                )
                # rr chain per pair
                if i % 2 == 1:
                    p = i // 2
                    nc.vector.tensor_reduce(
                        out=R2, in_=part_a[:, 2 * p:2 * p + 2, :],
                        axis=mybir.AxisListType.X, op=ALU.add,
                    )
                    nc.vector.tensor_tensor(
                        out=R2, in0=R2, in1=part_d[:, 2 * p:2 * p + 2],
                        op=ALU.add,
                    )
                    nc.vector.reciprocal(out=RR2, in_=R2)
                    for h2 in range(2):
                        nc.vector.tensor_scalar(
                            out=rmat[p % 2][:, h2, 0:577:192],
                            in0=zero4,
                            scalar1=RR2[:, h2:h2 + 1], scalar2=SIGMA,
                            op0=ALU.add, op1=ALU.mult,
                        )

            # last pair's colsum
            for k in range(4):
                emit_colsum_k(RT // 2 - 1, k)

            # ---- tail ----
            # w (x SIGMA*N) -> bf16
            nc.scalar.activation(out=wbb, in_=csum, func=AF.Identity)
            # replicate each 32-partition slice to all 128 partitions into
            # 4 distinct psum regions (slots 0-2 + freed csum banks), then
            # the 4 y-accumulations run back-to-back on DVE with no stalls
            wrep_t = [ringA[:, 0, :], ringA[:, 1, :], ring2, csum]
            for k in range(4):
                for h2 in range(2):
                    nc.tensor.matmul(
                        wrep_t[k][:, h2 * 512:(h2 + 1) * 512],
                        ones_sb[32 * k:32 * k + 1, :],
                        wbb[32 * k:32 * k + 1, h2 * 512:(h2 + 1) * 512],
                        start=True, stop=True,
                        tile_position=(32 * k, 0),
                    )
            for k in range(4):
                nc.vector.scalar_tensor_tensor(
                    out=ysc,
                    in0=xt_q[k], scalar=1.0, in1=wrep_t[k],
                    op0=ALU.mult, op1=ALU.mult,
                    accum_out=ycell[:, k:k + 1],
                )
            nc.vector.tensor_reduce(
                out=yv, in_=ycell, axis=mybir.AxisListType.X, op=ALU.add
            )
            # scale by 1/(N*SIGMA) during f32 keep
            nc.vector.tensor_scalar(
                out=yv, in0=yv, scalar1=1.0 / (N * SIGMA), scalar2=None,
                op0=ALU.mult,
            )
            # out[d] = sum_e WvT[e, d] * y[e] + 0.5*bv  (fp32 matmul)
            op = ring2[:, 0:1]
            nc.tensor.matmul(op, wv_sb, yv, start=True, stop=True)
            nc.scalar.activation(
                out=t32[:, 0:1], in_=op, func=AF.Identity, bias=bvs
            )
            nc.vector.transpose(out=t32t, in_=t32)
            nc.sync.dma_start(out[:, :], t32t[0:128:32, :])

    nc.compile()
    return nc


_cache = {}


def get_nc():
    if "nc" not in _cache:
        _cache["nc"] = build_nc()
    return _cache["nc"]


def make_in_maps(x, Wq, bq, Wk, bk, Wv, bv):
    x = np.asarray(x, np.float32)
    wqT = np.ascontiguousarray((A8 * np.asarray(Wq, np.float32)).T.astype(NPBF))
    wkT = np.ascontiguousarray(np.asarray(Wk, np.float32).T.astype(NPBF))
    wvT = np.ascontiguousarray(np.asarray(Wv, np.float32).T.astype(np.float32))
    bqc = np.ascontiguousarray(A8 * np.asarray(bq, np.float32).reshape(D, 1))
    bkc = np.ascontiguousarray(np.asarray(bk, np.float32).reshape(D, 1))
    bvc = np.ascontiguousarray(0.5 * np.asarray(bv, np.float32).reshape(D, 1))
    in_maps = []
    for c in range(NCORES):
        b = c // 2
        h = c % 2
        xbT = np.ascontiguousarray(x[b].T.astype(NPBF))  # [128, 4096] bf16
        in_maps.append(
            {
                "xt": xbT,
                "xq": np.ascontiguousarray(xbT[:, h * HALF:(h + 1) * HALF]),
                "wqT": wqT,
                "wkT": wkT,
                "wvT": wvT,
                "bq": bqc,
                "bk": bkc,
                "bvh": bvc,
            }
        )
    return in_maps


def combine(results):
    outs = [np.asarray(results[c]["out"]).reshape(D) for c in range(NCORES)]
    return np.stack([outs[2 * b] + outs[2 * b + 1] for b in range(B)]).astype(
        np.float32
    )


def run(inputs, trace=False, **kwargs):
    from concourse.bass_utils import run_bass_kernel_spmd

    nc = get_nc()
    in_maps = make_in_maps(**inputs)
    res = run_bass_kernel_spmd(
        nc, in_maps, core_ids=list(range(NCORES)), trace=trace, **kwargs
    )
    return combine(res.results), res


def kernel(x, Wq, bq, Wk, bk, Wv, bv):
    out, _ = run(dict(x=x, Wq=Wq, bq=bq, Wk=Wk, bk=bk, Wv=Wv, bv=bv))
    return out


# revision 20
# speedup vs baseline: 1.5781x; 1.0284x over previous
"""AttentionAggregation kernel for 8 TRN2 NeuronCores (v2 restructure).

Math: out[b] = mean_n softmax(Q K^T)[n,:] @ V  with Q/K/V = x @ W^T + b.
Fold: out[b,d] = sum_m w[b,m] V[b,m,d],  w[b,m] = (1/N) sum_n E[n,m]/R[n],
E = exp(S - c), R[n] = rowsum E.  Fixed shift c sized so E fits fp8e5 range.
V is never materialized: out = (w @ x) @ Wv^T + 0.5*bv (since sum_m w[m] = 1/2
per core), so the V projection and its eviction disappear from the main loop.

Structure per core (batch b = c//2, row half h = c%2, 16 row tiles of 128):
- PSUM: banks 0-5 = three 1024-col slots for S groups; banks 6-7 = colsum
  accumulator, live across all 8 pairs (no per-pair evictions).  Slot
  rotation alternates r in {0,1} per tile: writes (g0,g1,g2,g3) ->
  (r, r+1, r+2 mod 3, r); the ACT exp pair always sees a contiguous
  ascending 2048-col window and every slot's next writer follows the
  reader that frees it.
- Exp split: groups 0-1 on ACT (one 2048-wide exp + accum for R), groups
  2-3 Schraudolph on DVE (int8-bitcast e5m2 bits; host pre-scales Wq/bq by
  A8=4/ln2), row-sum re-read of groups 2-3 on Pool (one 2048-wide
  tensor_reduce into an R cell).  Slots only ever hold S'/projection values
  (<= ~117 in e5m2-bit units), so a stale int8 convert can never saturate
  into the e5m2 NaN code 127.
- rr = 1/R per pair: one reduce + reciprocal + fp8 cast writing col 0 of a
  shared [128,2,32] DoubleRow stationary; colsum matmuls place m-slice k on
  out partitions 32k..32k+31 via tile_position=(0,32k), so no per-slice
  stationary copies are needed.
- Colsum: fp8e5 DoubleRow, 4 compound 1024-wide matmuls per pair (K=256),
  accumulated in PSUM banks 6-7 across all pairs; rr*64 keeps e5m2 normal
  range; 1/64 and 1/N fold into the tail.
- Tail: evict colsum -> w bf16, replicate each 32-partition slice to all
  128 partitions (4 matmuls), y = sum_m w[m] x[m,:] via scalar_tensor_tensor
  with accum (DVE+Pool, 2 slices each), out = WvT^T y + 0.5 bv in fp32, then
  the 32x32 block transpose so the result leaves as 4 x 128B DMA packets.

Numerics: rel err ~1.2e-2 vs the 2e-2 gate (numpy-sim validated).
Sharding: core c handles batch b=c//2, row half h=c%2; host sums the two
per-core partial outputs per batch.
"""

import sys

sys.path.insert(0, "/opt/trn_rl_repo")

import ml_dtypes
import numpy as np

import concourse.bass as bass
import concourse.mybir as mybir
import concourse.tile as tile
from concourse import bacc

D = 128
N = 4096
B = 4
NCORES = 8
HALF = N // 2
RT = HALF // 128  # 16 row tiles per core
GW = 1024

C_SHIFT = 13.75
A8 = 4.0 / np.log(2.0)
B8 = 60.5 - C_SHIFT * A8
SIGMA = 64.0

F32 = mybir.dt.float32
BF16 = mybir.dt.bfloat16
FP8 = mybir.dt.float8e5
NPBF = ml_dtypes.bfloat16
AF = mybir.ActivationFunctionType
ALU = mybir.AluOpType
DRM = mybir.MatmulPerfMode.DoubleRow


def build_nc():
    nc = bacc.Bacc()
    xt = nc.dram_tensor("xt", [D, N], BF16, kind="ExternalInput")  # x[b].T
    xq = nc.dram_tensor("xq", [D, HALF], BF16, kind="ExternalInput")  # row half
    wqT = nc.dram_tensor("wqT", [D, D], BF16, kind="ExternalInput")  # A8*Wq.T
    wkT = nc.dram_tensor("wkT", [D, D], BF16, kind="ExternalInput")
    wvT = nc.dram_tensor("wvT", [D, D], F32, kind="ExternalInput")
    bq = nc.dram_tensor("bq", [D, 1], F32, kind="ExternalInput")  # A8*bq
    bk = nc.dram_tensor("bk", [D, 1], F32, kind="ExternalInput")
    bvh = nc.dram_tensor("bvh", [D, 1], F32, kind="ExternalInput")  # bv*0.5
    out = nc.dram_tensor("out", [4, 32], F32, kind="ExternalOutput")

    with tile.TileContext(nc) as tc:
        with (
            tc.tile_pool(name="singles", bufs=1) as singles,
            tc.tile_pool(name="ps", bufs=1, space="PSUM") as ps,
        ):
            # ---- SBUF ----
            wq_sb = singles.tile([D, D], BF16, tag="wq", name="wq_sb")
            wk_sb = singles.tile([D, D], BF16, tag="wk", name="wk_sb")
            wv_sb = singles.tile([D, D], F32, tag="wv", name="wv_sb")
            bqs = singles.tile([D, 1], F32, tag="bq", name="bqs")
            bks = singles.tile([D, 1], F32, tag="bk", name="bks")
            bvs = singles.tile([D, 1], F32, tag="bv", name="bvs")
            xt_q = [singles.tile([D, GW], BF16, tag=f"xt{q}", name=f"xt{q}")
                    for q in range(4)]
            xq_q = [singles.tile([D, GW], BF16, tag=f"xq{q}", name=f"xq{q}")
                    for q in range(2)]
            # K/Q in 512-col half tiles so ACT/DVE evictions and S matmuls
            # track dependencies per half (no write-write false ordering)
            kt_h = [singles.tile([D, 512], BF16, tag=f"kt{h}", name=f"kt{h}")
                    for h in range(8)]
            qt_h = [singles.tile([D, 512], BF16, tag=f"qt{h}", name=f"qt{h}")
                    for h in range(4)]
            # E split per writer: ACT pair (cols 0-2048), DVE schraudolph
            # (2048-3072), ACT single (3072-4096) -- separate tiles so the
            # three writers never order against each other
            E_a = [singles.tile([128, 2, 2 * GW], FP8, tag=f"Ea{b_}",
                                name=f"Ea{b_}") for b_ in range(2)]
            E_d = [singles.tile([128, 2, GW], FP8, tag=f"Ed{b_}",
                                name=f"Ed{b_}") for b_ in range(2)]
            E_b = [singles.tile([128, 2, GW], FP8, tag=f"Eb{b_}",
                                name=f"Eb{b_}") for b_ in range(2)]
            # DR stationary per pair: 4 windows of 160 cols; window k =
            # [160k, 160k+128), rr*SIGMA on its local col 32k (flat 192k)
            rmat = [singles.tile([128, 2, 640], FP8, tag=f"rm{b_}",
                                 name=f"rm{b_}") for b_ in range(2)]
            zero4 = singles.tile([128, 4], F32, tag="z4", name="zero4")
            part_a = singles.tile([128, RT, 2], F32, tag="parta", name="parta")
            part_d = singles.tile([128, RT], F32, tag="partd", name="partd")
            R2 = singles.tile([128, 2], F32, tag="R2", name="R2")
            RR2 = singles.tile([128, 2], F32, tag="RR2", name="RR2")
            cshift = singles.tile([128, 1], F32, tag="csh", name="cshift")
            ones_sb = singles.tile([D, D], BF16, tag="ones", name="ones_sb")
            wbb = singles.tile([128, GW], BF16, tag="wbb", name="wbb")
            ysc = singles.tile([128, GW], F32, tag="ysc", name="ysc")
            ycell = singles.tile([128, 4], F32, tag="ycell", name="ycell")
            yv = singles.tile([128, 1], F32, tag="yv", name="yv")
            t32 = singles.tile([128, 32], F32, tag="t32", name="t32")
            t32t = singles.tile([128, 32], F32, tag="t32t", name="t32t")
            rdump = singles.tile([128, GW], FP8, tag="rdump", name="rdump")

            nc.vector.memset(part_a, 0.0)
            nc.vector.memset(part_d, 0.0)
            nc.vector.memset(cshift, -C_SHIFT)
            nc.vector.memset(t32, 0.0)
            nc.vector.memset(ones_sb, 1.0)
            nc.vector.memset(zero4, 0.0)
            nc.gpsimd.memset(rmat[0], 0.0)
            nc.gpsimd.memset(rmat[1], 0.0)

            # ---- PSUM ----
            # slots 0,1 share one 4-bank tile (the ACT pair exp reads the
            # whole thing, so whole-tile WAR tracking is exact); slot 2 and
            # the colsum accumulator are their own tiles
            ringA = ps.tile([128, 2, GW], F32, tag="ringA", name="ringA")
            ring2 = ps.tile([128, GW], F32, tag="ring2", name="ring2")
            csum = ps.tile([128, GW], F32, tag="csum", name="csum")

            # ---- DMA prologue (3 queues, x quarters first in line) ----
            nc.sync.dma_start(wk_sb, wkT[:, :])
            nc.scalar.dma_start(wq_sb, wqT[:, :])
            nc.gpsimd.dma_start(bks, bk[:, :])
            nc.sync.dma_start(xt_q[0], xt[:, 0:GW])
            nc.scalar.dma_start(bqs, bq[:, :])
            nc.gpsimd.dma_start(xq_q[0], xq[:, 0:GW])
            nc.scalar.dma_start(xt_q[1], xt[:, GW:2 * GW])
            nc.gpsimd.dma_start(xt_q[2], xt[:, 2 * GW:3 * GW])
            nc.sync.dma_start(xq_q[1], xq[:, GW:2 * GW])
            nc.scalar.dma_start(xt_q[3], xt[:, 3 * GW:4 * GW])
            nc.gpsimd.dma_start(wv_sb, wvT[:, :])
            nc.scalar.dma_start(bvs, bvh[:, :])

            # ---- projections: K all 4 quarters, Q both halves ----
            # 512-granular: each half is its own SBUF tile, evicted on
            # ACT/DVE in parallel the moment its matmul lands
            def slot_ap(slot):
                return ringA[:, slot, :] if slot < 2 else ring2

            def proj(dsts, w_sb, src_sb, slot, bias_sb):
                pt = slot_ap(slot)
                for h2 in range(2):
                    nc.tensor.matmul(
                        pt[:, h2 * 512:(h2 + 1) * 512], w_sb,
                        src_sb[:, h2 * 512:(h2 + 1) * 512],
                        start=True, stop=True,
                    )
                    if h2 == 0:
                        nc.scalar.activation(
                            out=dsts[0], in_=pt[:, 0:512],
                            func=AF.Identity, bias=bias_sb,
                        )
                    else:
                        nc.vector.tensor_scalar(
                            out=dsts[1], in0=pt[:, 512:1024],
                            scalar1=bias_sb, scalar2=None, op0=ALU.add,
                        )

            # regions: ringA (slot arg 0/1 share the tile), ring2, csum --
            # spread the 6 projections over the 3 independent regions
            def proj_c(dsts, w_sb, src_sb, bias_sb):
                for h2 in range(2):
                    nc.tensor.matmul(
                        csum[:, h2 * 512:(h2 + 1) * 512], w_sb,
                        src_sb[:, h2 * 512:(h2 + 1) * 512],
                        start=True, stop=True,
                    )
                    if h2 == 0:
                        nc.scalar.activation(
                            out=dsts[0], in_=csum[:, 0:512],
                            func=AF.Identity, bias=bias_sb,
                        )
                    else:
                        nc.vector.tensor_scalar(
                            out=dsts[1], in0=csum[:, 512:1024],
                            scalar1=bias_sb, scalar2=None, op0=ALU.add,
                        )

            proj(kt_h[0:2], wk_sb, xt_q[0], 0, bks)
            proj(qt_h[0:2], wq_sb, xq_q[0], 2, bqs)
            proj_c(kt_h[2:4], wk_sb, xt_q[1], bks)
            proj(kt_h[4:6], wk_sb, xt_q[2], 1, bks)
            proj(kt_h[6:8], wk_sb, xt_q[3], 0, bks)
            proj(qt_h[2:4], wq_sb, xq_q[1], 2, bqs)

            # ---- main loop ----
            def s_mm(slot, lhsT, q):
                # matmul out is ISA-capped at 512 (one PSUM bank)
                pt = slot_ap(slot)
                for h2 in range(2):
                    nc.tensor.matmul(
                        pt[:, h2 * 512:(h2 + 1) * 512],
                        lhsT,
                        kt_h[2 * q + h2],
                        start=True, stop=True,
                    )

            def emit_colsum_k(p, k):
                # matmul rhs free size is ISA-capped at 1024 -> 2x512 halves
                if k < 2:
                    esrc = lambda h2: E_a[p % 2][:, :, k * GW + h2 * 512:
                                                 k * GW + (h2 + 1) * 512]
                elif k == 2:
                    esrc = lambda h2: E_d[p % 2][:, :, h2 * 512:(h2 + 1) * 512]
                else:
                    esrc = lambda h2: E_b[p % 2][:, :, h2 * 512:(h2 + 1) * 512]
                for h2 in range(2):
                    nc.tensor.matmul(
                        csum[:, h2 * 512:(h2 + 1) * 512],
                        rmat[p % 2][:, :, 160 * k:160 * k + 128],
                        esrc(h2),
                        start=(p == 0 and k == 0),
                        stop=(p == RT // 2 - 1 and k == 3),
                        perf_mode=DRM,
                        skip_group_check=True,
                    )

            for i in range(RT):
                # fixed slots: s0,s1 feed the ACT 2048 pair exp; s2 is used
                # twice (DVE schraudolph group, then ACT 1024 group)
                hh = i % 2
                bb = (i // 2) % 2
                lhsT = qt_h[i // 4][:, (i % 4) * 128:(i % 4) * 128 + 128]

                s_mm(0, lhsT, 0)
                s_mm(1, lhsT, 1)
                s_mm(2, lhsT, 2)
                # ACT: pair exp over slots 0,1 (banks 0-3, contiguous)
                nc.scalar.activation(
                    out=E_a[bb][:, hh, :],
                    in_=ringA[:, :, :],
                    func=AF.Exp,
                    bias=cshift,
                    scale=1.0 / A8,
                    accum_out=part_a[:, i, 0:1],
                )
                # DVE: schraudolph bits for cols 2048-3072 (slot 2)
                nc.vector.tensor_scalar(
                    out=E_d[bb][:, hh, :].bitcast(mybir.dt.int8),
                    in0=ring2,
                    scalar1=B8, scalar2=0.0,
                    op0=ALU.add, op1=ALU.max,
                )
                # S group 3 reuses slot 2 once the bits pass freed it
                s_mm(2, lhsT, 3)
                # colsum of pair p=(i-2)//2 goes AFTER g3 on the tensor
                # queue: its LDW may wait on the rr stationary and must not
                # block g3 (which gates the ACT 1024-exp)
                if i >= 2:
                    p = (i - 2) // 2
                    if i % 2 == 0:
                        emit_colsum_k(p, 0)
                        emit_colsum_k(p, 1)
                    else:
                        emit_colsum_k(p, 2)
                        emit_colsum_k(p, 3)
                # DVE: row-sum re-read of the schraudolph group
                nc.vector.tensor_scalar(
                    out=rdump,
                    in0=E_d[bb][:, hh, :],
                    scalar1=0.0, scalar2=0.0,
                    op0=ALU.add, op1=ALU.add,
                    accum_out=part_d[:, i:i + 1],
                )
                # ACT: exp of cols 3072-4096 (slot 2 again)
                nc.scalar.activation(
                    out=E_b[bb][:, hh, :],
                    in_=ring2,
                    func=AF.Exp,
                    bias=cshift,
                    scale=1.0 / A8,
                    accum_out=part_a[:, i, 1:2],
                )
                # rr chain per pair
                if i % 2 == 1:
                    p = i // 2
                    nc.vector.tensor_reduce(
                        out=R2, in_=part_a[:, 2 * p:2 * p + 2, :],
                        axis=mybir.AxisListType.X, op=ALU.add,
                    )
                    nc.vector.tensor_tensor(
                        out=R2, in0=R2, in1=part_d[:, 2 * p:2 * p + 2],
                        op=ALU.add,
                    )
                    nc.vector.reciprocal(out=RR2, in_=R2)
                    for h2 in range(2):
                        nc.vector.tensor_scalar(
                            out=rmat[p % 2][:, h2, 0:577:192],
                            in0=zero4,
                            scalar1=RR2[:, h2:h2 + 1], scalar2=SIGMA,
                            op0=ALU.add, op1=ALU.mult,
                        )

            # keep the PE p-state warm while waiting for the last rr
            for wz in range(6):
                nc.tensor.matmul(
                    ringA[:, 0, 0:512], wk_sb, kt_h[wz % 8],
                    start=True, stop=True,
                )
            # last pair's colsum
            for k in range(4):
                emit_colsum_k(RT // 2 - 1, k)

            # ---- tail ----
            # w (x SIGMA*N) -> bf16
            nc.scalar.activation(out=wbb, in_=csum, func=AF.Identity)
            # replicate each 32-partition slice to all 128 partitions into
            # 4 distinct psum regions (slots 0-2 + freed csum banks), then
            # the 4 y-accumulations run back-to-back on DVE with no stalls
            wrep_t = [ringA[:, 0, :], ringA[:, 1, :], ring2, csum]
            for k in range(4):
                for h2 in range(2):
                    nc.tensor.matmul(
                        wrep_t[k][:, h2 * 512:(h2 + 1) * 512],
                        ones_sb[32 * k:32 * k + 1, :],
                        wbb[32 * k:32 * k + 1, h2 * 512:(h2 + 1) * 512],
                        start=True, stop=True,
                        tile_position=(32 * k, 0),
                    )
            for k in range(4):
                nc.vector.scalar_tensor_tensor(
                    out=ysc,
                    in0=xt_q[k], scalar=1.0, in1=wrep_t[k],
                    op0=ALU.mult, op1=ALU.mult,
                    accum_out=ycell[:, k:k + 1],
                )
            nc.vector.tensor_reduce(
                out=yv, in_=ycell, axis=mybir.AxisListType.X, op=ALU.add
            )
            # scale by 1/(N*SIGMA) during f32 keep
            nc.vector.tensor_scalar(
                out=yv, in0=yv, scalar1=1.0 / (N * SIGMA), scalar2=None,
                op0=ALU.mult,
            )
            # out[d] = sum_e WvT[e, d] * y[e] + 0.5*bv  (fp32 matmul)
            op = ring2[:, 0:1]
            nc.tensor.matmul(op, wv_sb, yv, start=True, stop=True)
            nc.scalar.activation(
                out=t32[:, 0:1], in_=op, func=AF.Identity, bias=bvs
            )
            nc.vector.transpose(out=t32t, in_=t32)
            nc.sync.dma_start(out[:, :], t32t[0:128:32, :])

    nc.compile()
    return nc


_cache = {}


def get_nc():
    if "nc" not in _cache:
        _cache["nc"] = build_nc()
    return _cache["nc"]


def make_in_maps(x, Wq, bq, Wk, bk, Wv, bv):
    x = np.asarray(x, np.float32)
    wqT = np.ascontiguousarray((A8 * np.asarray(Wq, np.float32)).T.astype(NPBF))
    wkT = np.ascontiguousarray(np.asarray(Wk, np.float32).T.astype(NPBF))
    wvT = np.ascontiguousarray(np.asarray(Wv, np.float32).T.astype(np.float32))
    bqc = np.ascontiguousarray(A8 * np.asarray(bq, np.float32).reshape(D, 1))
    bkc = np.ascontiguousarray(np.asarray(bk, np.float32).reshape(D, 1))
    bvc = np.ascontiguousarray(0.5 * np.asarray(bv, np.float32).reshape(D, 1))
    in_maps = []
    for c in range(NCORES):
        b = c // 2
        h = c % 2
        xbT = np.ascontiguousarray(x[b].T.astype(NPBF))  # [128, 4096] bf16
        in_maps.append(
            {
                "xt": xbT,
                "xq": np.ascontiguousarray(xbT[:, h * HALF:(h + 1) * HALF]),
                "wqT": wqT,
                "wkT": wkT,
                "wvT": wvT,
                "bq": bqc,
                "bk": bkc,
                "bvh": bvc,
            }
        )
    return in_maps


def combine(results):
    outs = [np.asarray(results[c]["out"]).reshape(D) for c in range(NCORES)]
    return np.stack([outs[2 * b] + outs[2 * b + 1] for b in range(B)]).astype(
        np.float32
    )


def run(inputs, trace=False, **kwargs):
    from concourse.bass_utils import run_bass_kernel_spmd

    nc = get_nc()
    in_maps = make_in_maps(**inputs)
    res = run_bass_kernel_spmd(
        nc, in_maps, core_ids=list(range(NCORES)), trace=trace, **kwargs
    )
    return combine(res.results), res


def kernel(x, Wq, bq, Wk, bk, Wv, bv):
    out, _ = run(dict(x=x, Wq=Wq, bq=bq, Wk=Wk, bk=bk, Wv=Wv, bv=bv))
    return out
